# revision 89
# baseline (speedup 1.0000x reference)
"""Trainium2 Bass kernel for complex-valued spatial-reduction attention.

x: [B=4, N=2304, C=512] complex64 (re/im f32 planes), H=W=48, 8 heads,
head_dim 64, sr_ratio 2 -> Nk=576.

Sharding: 8 cores = 4 batches x 2 head-groups (4 heads each). Each core:
sr-conv over full C, complex LayerNorm, q/k/v for its heads,
softmax(|q.k^T|) attention, attn @ v, partial output projection.
Host sums the two partials per batch and adds bproj.

Layout: q and k projections write PSUM in per-head packed form
[re(64) ; im(64)] x tokens via host-packed weight planes
[Wr_h|Wi_h] / [-Wi_h|Wr_h], so each complex score needs ONE K=128
matmul instead of two K=64 ones.  The whole attention tail (q-proj ->
scores -> softmax -> attn@v -> normalize -> out-proj) is fused per
2304/5-column chunk with no DRAM roundtrips, software-pipelined so the
PE runs chunk j+1's scores while chunk j's softmax occupies ACT/DVE.
Softmax |a| uses s=re^2+im^2 (ACT square + DVE copy/mul/add), then
batched Ln and Exp waves (|a| = exp(0.5*ln s); the HW Sqrt LUT was
measured too inaccurate) so the ACT table is swapped only twice per
chunk.  Big DMA streams alternate between the SP, ACT, and gpsimd
issue queues, each of which is a serial ~358GB/s channel.
"""

import os
import contextlib

import numpy as np
import ml_dtypes

import concourse.bass as bass
import concourse.mybir as mybir
import concourse.tile as tile
from concourse import bacc
from concourse.masks import make_identity

BF16 = mybir.dt.bfloat16
F16 = mybir.dt.float16
F32 = mybir.dt.float32
F32R = mybir.dt.float32r
AF = mybir.ActivationFunctionType
ALU = mybir.AluOpType

B, N, C, HEADS, HD, SR = 4, 2304, 512, 8, 64, 2
NK = 576
HR = 24
EPS = 1e-5
SCALE = HD ** -0.5  # folded into Wk host-side

K_CHUNKS = [(0, 128), (128, 128), (256, 128), (384, 128), (512, 64)]
Q_CHUNKS = [(0, 512), (512, 512), (1024, 512), (1536, 512), (2048, 256)]

DEBUG = bool(int(os.environ.get("KBUILD_DEBUG", "0")))


def _r(ap):
    return ap.bitcast(F32R)


def build_nc():
    nc = bacc.Bacc("TRN2", target_bir_lowering=False, debug=False, num_devices=8)

    xT_d = nc.dram_tensor("xT", [2, 128, 4, N], F32R, kind="ExternalInput")
    xP_d = nc.dram_tensor("xP", [16, 128, 2 * NK], F32R, kind="ExternalInput")
    wc_d = nc.dram_tensor("wc", [128, 16 * 3 * 512], F32R,
                          kind="ExternalInput")
    srb_d = nc.dram_tensor("srb", [2, C], F32R, kind="ExternalInput")
    ones_d = nc.dram_tensor("ones", [1, 512], F32R, kind="ExternalInput")
    wq_d = nc.dram_tensor("wq", [128, 4 * 2 * 512], F32R, kind="ExternalInput")
    wk_d = nc.dram_tensor("wk", [128, 4 * 2 * 512], F32R, kind="ExternalInput")
    bk_d = nc.dram_tensor("bk", [1, 512], F32R, kind="ExternalInput")
    wv_d = nc.dram_tensor("wv", [128, 4 * 3 * 256], F32R, kind="ExternalInput")
    bv_d = nc.dram_tensor("bv", [2, 256], F32R, kind="ExternalInput")
    wp_d = nc.dram_tensor("wp", [128, 4 * 2 * C], BF16, kind="ExternalInput")
    outT_d = nc.dram_tensor("outT", [2, 4, 128, N], F16, kind="ExternalOutput")
    dbg = {}
    if DEBUG:
        dbg["xnT"] = nc.dram_tensor("dbg_xnT", [2, C, NK], F32, kind="ExternalOutput")
        dbg["q"] = nc.dram_tensor("dbg_q", [4, 128, N], F32, kind="ExternalOutput")
        dbg["kre"] = nc.dram_tensor("dbg_kre", [4, 128, NK], F32, kind="ExternalOutput")
        dbg["kim"] = nc.dram_tensor("dbg_kim", [4, 128, NK], F32, kind="ExternalOutput")
        dbg["v"] = nc.dram_tensor("dbg_v", [128, 5 * 4 * 128], BF16, kind="ExternalOutput")
        dbg["dn"] = nc.dram_tensor("dbg_dn", [4, N], F32, kind="ExternalOutput")

    with tile.TileContext(nc) as tc:
        _body(nc, tc, xT_d, xP_d, wc_d, srb_d, ones_d, wq_d, wk_d, bk_d,
              wv_d, bv_d, wp_d, outT_d, dbg)

    nc.compile()
    return nc


def _ln_chunk(nc, work, stats, re_sb, im_sb, sz):
    """Complex LayerNorm for one [sz, C] chunk (inputs in SBUF) -> (xnr, xni).

    Stats sums run on ACT via accum_out (the conv phase leaves ACT idle);
    only the cross-product sum needs a DVE tensor_tensor_reduce."""
    inv_c = 1.0 / C
    sum_r = stats.tile([128, 1], F32, tag="sum_r")
    sum_i = stats.tile([128, 1], F32, tag="sum_i")
    junk = work.tile([128, C], F32, tag="ln_junk", bufs=1)
    junk2 = work.tile([128, C], F32, tag="ln_junk2", bufs=1)
    sxx = stats.tile([128, 1], F32, tag="sxx")
    sii = stats.tile([128, 1], F32, tag="sii")
    sxi = stats.tile([128, 1], F32, tag="sxi")
    nc.vector.tensor_reduce(sum_r[:sz], re_sb[:sz], mybir.AxisListType.X, ALU.add)
    nc.vector.tensor_reduce(sum_i[:sz], im_sb[:sz], mybir.AxisListType.X, ALU.add)
    nc.vector.tensor_mul(junk[:sz], re_sb[:sz], re_sb[:sz])
    nc.vector.tensor_reduce(sxx[:sz], junk[:sz], mybir.AxisListType.X, ALU.add)
    nc.vector.tensor_mul(junk2[:sz], im_sb[:sz], im_sb[:sz])
    nc.vector.tensor_reduce(sii[:sz], junk2[:sz], mybir.AxisListType.X, ALU.add)
    nc.vector.tensor_mul(junk[:sz], re_sb[:sz], im_sb[:sz])
    nc.vector.tensor_reduce(sxi[:sz], junk[:sz], mybir.AxisListType.X, ALU.add)
    mr = stats.tile([128, 1], F32, tag="mr")
    mi = stats.tile([128, 1], F32, tag="mi")
    nc.vector.tensor_scalar_mul(mr[:sz], sum_r[:sz], inv_c)
    nc.vector.tensor_scalar_mul(mi[:sz], sum_i[:sz], inv_c)
    vre = stats.tile([128, 1], F32, tag="vre")
    vim = stats.tile([128, 1], F32, tag="vim")
    tA = stats.tile([128, 1], F32, tag="tA")
    tB = stats.tile([128, 1], F32, tag="tB")
    nc.vector.tensor_sub(tA[:sz], sxx[:sz], sii[:sz])
    nc.vector.tensor_scalar_mul(tA[:sz], tA[:sz], inv_c)
    nc.vector.tensor_mul(vre[:sz], mr[:sz], mr[:sz])
    nc.vector.tensor_mul(tB[:sz], mi[:sz], mi[:sz])
    nc.vector.tensor_sub(vre[:sz], vre[:sz], tB[:sz])
    nc.vector.tensor_sub(vre[:sz], tA[:sz], vre[:sz])
    nc.vector.tensor_scalar_add(vre[:sz], vre[:sz], EPS)
    nc.vector.tensor_mul(tB[:sz], mr[:sz], mi[:sz])
    nc.vector.tensor_scalar_mul(tB[:sz], tB[:sz], 2.0)
    nc.vector.tensor_scalar_mul(vim[:sz], sxi[:sz], 2.0 * inv_c)
    nc.vector.tensor_sub(vim[:sz], vim[:sz], tB[:sz])
    r2 = stats.tile([128, 1], F32, tag="r2")
    nc.vector.tensor_mul(r2[:sz], vre[:sz], vre[:sz])
    nc.vector.tensor_mul(tB[:sz], vim[:sz], vim[:sz])
    nc.vector.tensor_add(r2[:sz], r2[:sz], tB[:sz])

    def _sqrt_newton(out, x, sc):
        # y0 = LUT sqrt(sc*x); y1 = 0.5*(y0 + sc*x/y0)  (one Newton step)
        y0 = stats.tile([128, 1], F32, tag="nw_y0")
        nc.scalar.activation(y0[:sz], x[:sz], AF.Sqrt, scale=sc)
        yr = stats.tile([128, 1], F32, tag="nw_yr")
        nc.vector.tensor_scalar_add(y0[:sz], y0[:sz], 1e-30)
        nc.vector.reciprocal(yr[:sz], y0[:sz])
        nc.vector.tensor_mul(yr[:sz], yr[:sz], x[:sz])
        if sc != 1.0:
            nc.vector.tensor_scalar_mul(yr[:sz], yr[:sz], sc)
        nc.vector.tensor_add(out[:sz], y0[:sz], yr[:sz])
        nc.vector.tensor_scalar_mul(out[:sz], out[:sz], 0.5)

    rr = stats.tile([128, 1], F32, tag="rr")
    _sqrt_newton(rr, r2, 1.0)
    srt = stats.tile([128, 1], F32, tag="srt")
    sia = stats.tile([128, 1], F32, tag="sia")
    nc.vector.tensor_add(tA[:sz], rr[:sz], vre[:sz])
    _sqrt_newton(srt, tA, 0.5)
    nc.vector.tensor_sub(tA[:sz], rr[:sz], vre[:sz])
    _sqrt_newton(sia, tA, 0.5)
    sgn = stats.tile([128, 1], F32, tag="sgn")
    nc.scalar.activation(sgn[:sz], vim[:sz], AF.Sign)
    nc.vector.tensor_mul(sia[:sz], sia[:sz], sgn[:sz])
    rin = stats.tile([128, 1], F32, tag="rin")
    nc.vector.reciprocal(rin[:sz], rr[:sz])
    wr = stats.tile([128, 1], F32, tag="wr")
    wn = stats.tile([128, 1], F32, tag="wn")  # = -w_im
    nc.vector.tensor_mul(wr[:sz], srt[:sz], rin[:sz])
    nc.vector.tensor_mul(wn[:sz], sia[:sz], rin[:sz])
    aT = work.tile([128, C], F32, tag="ln_a", bufs=1)
    bT = work.tile([128, C], F32, tag="ln_b", bufs=1)
    xnr = work.tile([128, C], F32, tag="ln_xnr")
    xni = work.tile([128, C], F32, tag="ln_xni")
    nc.vector.tensor_scalar(aT[:sz], re_sb[:sz], mr[:sz], wr[:sz],
                            ALU.subtract, ALU.mult)
    nc.vector.tensor_scalar(bT[:sz], im_sb[:sz], mi[:sz], wn[:sz],
                            ALU.subtract, ALU.mult)
    nc.vector.tensor_add(xnr[:sz], aT[:sz], bT[:sz])
    nc.vector.tensor_scalar(aT[:sz], re_sb[:sz], mr[:sz], wn[:sz],
                            ALU.subtract, ALU.mult)
    nc.vector.tensor_scalar(bT[:sz], im_sb[:sz], mi[:sz], wr[:sz],
                            ALU.subtract, ALU.mult)
    nc.vector.tensor_sub(xni[:sz], bT[:sz], aT[:sz])
    return xnr, xni


def _body(nc, tc, xT_d, xP_d, wc_d, srb_d, ones_d, wq_d, wk_d, bk_d,
          wv_d, bv_d, wp_d, outT_d, dbg):
    ctx = contextlib.ExitStack()
    consts = ctx.enter_context(tc.tile_pool(name="consts", bufs=1))
    big = ctx.enter_context(tc.tile_pool(name="big", bufs=1))
    psum = ctx.enter_context(tc.tile_pool(name="psum", bufs=8, space="PSUM"))

    # ---- constants ----
    ident = consts.tile([128, 128], F32, tag="ident")
    make_identity(nc, ident)
    ones_col = consts.tile([128, 1], BF16, tag="ones_col")
    nc.vector.memset(ones_col, 1.0)
    ones_row = consts.tile([1, 512], F32R, tag="ones_row")
    nc.sync.dma_start(ones_row[:], ones_d[:])
    nbias = consts.tile([128, 1], F32, tag="nbias")
    nc.vector.memset(nbias, -50.0)

    srb_re = consts.tile([1, C], F32R, tag="srb_re")
    srb_im = consts.tile([1, C], F32R, tag="srb_im")
    nc.sync.dma_start(srb_re[:], srb_d[0:1, :])
    nc.sync.dma_start(srb_im[:], srb_d[1:2, :])
    bk_sb = consts.tile([1, 512], F32R, tag="bk_sb")
    nc.sync.dma_start(bk_sb[:], bk_d[:])
    bv_re = consts.tile([1, 256], F32R, tag="bv_re")
    bv_im = consts.tile([1, 256], F32R, tag="bv_im")
    nc.sync.dma_start(bv_re[:], bv_d[0:1, :])
    nc.sync.dma_start(bv_im[:], bv_d[1:2, :])

    # ---- persistent SBUF (whole kernel) ----
    kre = big.tile([128, 4, NK], F32R, tag="kre")   # [kr_h ; -ki_h]
    kim = big.tile([128, 4, NK], F32R, tag="kim")   # [ki_h ;  kr_h]
    vpk = big.tile([128, 5, 4, 128], BF16, tag="vpk")
    wq_sb = big.tile([128, 4, 2, 512], F32R, tag="wq_sb")
    nc.gpsimd.dma_start(wq_sb.rearrange("p a b c -> p (a b c)"), wq_d[:])
    if DEBUG:
        nc.vector.memset(vpk, 0.0)

    CGROUPS = [(0, 128), (128, 128), (256, 128), (384, 128), (512, 64)]

    qs = ctx.enter_context(tc.tile_pool(name="qs", bufs=2))

    def emit_qproj(q0, nq):
        xq = qs.tile([128, 4, 2, 512], F32R, tag="xq")
        nc.gpsimd.dma_start(xq[:, :, 0, :nq], xT_d[0, :, :, q0:q0 + nq])
        nc.sync.dma_start(xq[:, :, 1, :nq], xT_d[1, :, :, q0:q0 + nq])
        qsb = qs.tile([128, 4, 512], F32R, tag="qsb")
        for hpair in range(2):
            phs = []
            for j in range(2):
                h = 2 * hpair + j
                hs = slice(128 * h, 128 * (h + 1))
                ph = psum.tile([128, 512], F32, tag="bank", name=f"qp{h}")
                for cj in range(4):
                    nc.tensor.matmul(ph[:, :nq], wq_sb[:, cj, 0, hs],
                                     xq[:, cj, 0, :nq], start=cj == 0, stop=False)
                    nc.tensor.matmul(ph[:, :nq], wq_sb[:, cj, 1, hs],
                                     xq[:, cj, 1, :nq], start=False, stop=cj == 3)
                phs.append((h, ph))
            for (h, ph) in phs:
                nc.vector.tensor_copy(qsb[:, h, :nq], ph[:, :nq])
        if DEBUG:
            for h in range(4):
                nc.sync.dma_start(dbg["q"][h, :, q0:q0 + nq],
                                  qsb[:, h, :nq].bitcast(F32))
        return qsb

    # q-proj of chunk 0 issued before the conv: fills the PE during the
    # initial weight/x DMA waits.
    qsb0 = emit_qproj(*Q_CHUNKS[0])

    # =====================================================================
    # Phase 1+2 scope: conv + LayerNorm + k/v projections
    # =====================================================================
    with contextlib.ExitStack() as cvx:
        xs = cvx.enter_context(tc.tile_pool(name="xs", bufs=2))
        wcp = cvx.enter_context(tc.tile_pool(name="wcp", bufs=2))
        work = cvx.enter_context(tc.tile_pool(name="work", bufs=2))
        stats = cvx.enter_context(tc.tile_pool(name="stats", bufs=2))
        cbig = cvx.enter_context(tc.tile_pool(name="cbig", bufs=1))

        xnTr = cbig.tile([128, 4, NK], F32R, tag="xnTr")
        xnTi = cbig.tile([128, 4, NK], F32R, tag="xnTi")
        wk_sb = cbig.tile([128, 4, 2, 512], F32R, tag="wk_sb")
        wv_sb = cbig.tile([128, 4, 3, 256], F32R, tag="wv_sb")
        cvr = [cbig.tile([128, C], F32, tag=f"cvr{g}", name=f"cvr{g}")
               for g in range(5)]
        cvi = [cbig.tile([128, C], F32, tag=f"cvi{g}", name=f"cvi{g}")
               for g in range(5)]

        # ---- conv (f32r): two row-mega-group epochs (PSUM can hold at most
        # ~half the conv output), batched DMA loads ----
        MGS = [[0, 1, 2], [3, 4]]
        for mg in MGS:
            tg0 = CGROUPS[mg[0]][0]
            tgs = sum(CGROUPS[g][1] for g in mg)
            cps = {}
            for g in mg:
                cps[g] = (psum.tile([128, C], F32, tag="bank", name=f"cvr{g}"),
                          psum.tile([128, C], F32, tag="bank", name=f"cvi{g}"))
            for kk in range(16):
                xp = xs.tile([128, 2, 384], F32R, tag="xp")
                xp_v = xP_d[kk].rearrange("p (a b) -> p a b", a=2)
                nc.gpsimd.dma_start(xp[:, :, :tgs], xp_v[:, :, tg0:tg0 + tgs])
                # wc is 25MB total; one issuing engine is one serial DMA
                # channel, so alternate the two big streams between the SP
                # and ACT hwdge queues (ACT is idle during the conv).
                wcc = wcp.tile([128, 3, 512], F32R, tag="wcc")
                qw = nc.sync if kk % 2 == 0 else nc.scalar
                qw.dma_start(wcc.rearrange("p a b -> p (a b)"),
                             wc_d[:, 1536 * kk:1536 * (kk + 1)])
                for g in mg:
                    t0, sz = CGROUPS[g]
                    pat_r = xp[:, 0, t0 - tg0:t0 - tg0 + sz]
                    pat_i = xp[:, 1, t0 - tg0:t0 - tg0 + sz]
                    cre, cim = cps[g]
                    st = kk == 0
                    nc.tensor.matmul(cre[:sz, :], pat_r, wcc[:, 0, :],
                                     start=st, stop=False)
                    nc.tensor.matmul(cre[:sz, :], pat_i, wcc[:, 2, :],
                                     start=False, stop=False)
                    nc.tensor.matmul(cim[:sz, :], pat_r, wcc[:, 1, :],
                                     start=st, stop=False)
                    nc.tensor.matmul(cim[:sz, :], pat_i, wcc[:, 0, :],
                                     start=False, stop=False)
            for g in mg:
                t0, sz = CGROUPS[g]
                cre, cim = cps[g]
                nc.tensor.matmul(cre[:sz, :], _r(ones_row[:, :sz]),
                                 srb_re[:], start=False, stop=True)
                nc.tensor.matmul(cim[:sz, :], _r(ones_row[:, :sz]),
                                 srb_im[:], start=False, stop=True)
                nc.vector.tensor_copy(cvr[g][:sz, :], cre[:sz, :])
                nc.vector.tensor_copy(cvi[g][:sz, :], cim[:sz, :])

        def emit_ln(groups):
            for g in groups:
                t0, sz = CGROUPS[g]
                xnr, xni = _ln_chunk(nc, work, stats, cvr[g], cvi[g], sz)
                for cj in range(4):
                    for src, dst in ((xnr, xnTr), (xni, xnTi)):
                        pt = psum.tile([128, 128], F32, tag="bank")
                        nc.tensor.transpose(pt[:, :sz],
                                            src[:sz, 128 * cj:128 * (cj + 1)],
                                            ident[:sz, :sz])
                        nc.vector.tensor_copy(dst[:, cj, t0:t0 + sz],
                                              pt[:, :sz])

        def emit_kproj(n0, nn):
            for hpair in range(2):
                phs = []
                for j in range(2):
                    h = 2 * hpair + j
                    ph = psum.tile([128, 512], F32, tag="bank", name=f"kp{h}")
                    for cj in range(4):
                        hs = slice(128 * h, 128 * (h + 1))
                        nc.tensor.matmul(ph[:, :nn], wk_sb[:, cj, 0, hs],
                                         xnTr[:, cj, n0:n0 + nn],
                                         start=cj == 0, stop=False)
                        nc.tensor.matmul(ph[:, :nn], wk_sb[:, cj, 1, hs],
                                         xnTi[:, cj, n0:n0 + nn],
                                         start=False, stop=False)
                    nc.tensor.matmul(ph[:, :nn], bk_sb[:, 128 * h:128 * (h + 1)],
                                     _r(ones_row[:, :nn]), start=False, stop=True)
                    phs.append((h, ph))
                for (h, ph) in phs:
                    nc.vector.tensor_copy(kre[0:64, h, n0:n0 + nn], ph[0:64, :nn])
                    nc.vector.tensor_scalar_mul(kre[64:128, h, n0:n0 + nn],
                                                ph[64:128, :nn], -1.0)
                    nc.vector.tensor_copy(kim[0:64, h, n0:n0 + nn], ph[64:128, :nn])
                    nc.vector.tensor_copy(kim[64:128, h, n0:n0 + nn], ph[0:64, :nn])

        def emit_vproj(kcs):
            for kc in kcs:
                k0, szk = K_CHUNKS[kc]
                pr = psum.tile([128, 512], F32, tag="bank", name="vpr")
                pi = psum.tile([128, 512], F32, tag="bank", name="vpi")
                for cj in range(4):
                    st = cj == 0
                    nc.tensor.matmul(pr[:szk, :256], xnTr[:, cj, k0:k0 + szk],
                                     wv_sb[:, cj, 0, :], start=st, stop=False)
                    nc.tensor.matmul(pr[:szk, :256], xnTi[:, cj, k0:k0 + szk],
                                     wv_sb[:, cj, 2, :], start=False, stop=False)
                    nc.tensor.matmul(pi[:szk, :256], xnTr[:, cj, k0:k0 + szk],
                                     wv_sb[:, cj, 1, :], start=st, stop=False)
                    nc.tensor.matmul(pi[:szk, :256], xnTi[:, cj, k0:k0 + szk],
                                     wv_sb[:, cj, 0, :], start=False, stop=False)
                nc.tensor.matmul(pr[:szk, :256], _r(ones_row[:, :szk]),
                                 _r(bv_re[:]), start=False, stop=True)
                nc.tensor.matmul(pi[:szk, :256], _r(ones_row[:, :szk]),
                                 _r(bv_im[:]), start=False, stop=True)
                vr_v = pr[:szk, :256].rearrange("p (h d) -> p h d", h=4)
                vi_v = pi[:szk, :256].rearrange("p (h d) -> p h d", h=4)
                nc.vector.tensor_copy(vpk[:szk, kc, :, 0:64], vr_v)
                nc.vector.tensor_copy(vpk[:szk, kc, :, 64:128], vi_v)

        # k/v weights stream behind the conv data so the first conv kk
        # chunk isn't stuck behind 4MB of projection weights in the queue.
        nc.gpsimd.dma_start(wk_sb.rearrange("p a b c -> p (a b c)"), wk_d[:])
        nc.gpsimd.dma_start(wv_sb.rearrange("p a b c -> p (a b c)"), wv_d[:])

        # LN + kv interleaved: keys 0-287 and v-chunks 0-2 depend only on
        # conv groups 0-2, so their projections overlap the LN of groups 3-4.
        emit_ln((0, 1, 2))
        emit_kproj(0, 288)
        emit_vproj((0, 1, 2))
        emit_ln((3, 4))
        emit_kproj(288, 288)
        emit_vproj((3, 4))

        if DEBUG:
            for cj in range(4):
                nc.sync.dma_start(dbg["xnT"][0, 128 * cj:128 * (cj + 1), :],
                                  xnTr[:, cj, :].bitcast(F32))
                nc.sync.dma_start(dbg["xnT"][1, 128 * cj:128 * (cj + 1), :],
                                  xnTi[:, cj, :].bitcast(F32))

    if DEBUG:
        for h in range(4):
            nc.sync.dma_start(dbg["kre"][h], kre[:, h, :].bitcast(F32))
            nc.sync.dma_start(dbg["kim"][h], kim[:, h, :].bitcast(F32))
        nc.sync.dma_start(dbg["v"][:, :], vpk.rearrange("p a b c -> p (a b c)"))

    # =====================================================================
    # Phase 3: fused q-proj -> scores -> softmax -> attn@v -> norm -> proj
    # =====================================================================
    big2 = ctx.enter_context(tc.tile_pool(name="big2", bufs=1))
    sm = ctx.enter_context(tc.tile_pool(name="sm", bufs=2))
    st3 = ctx.enter_context(tc.tile_pool(name="st3", bufs=2))
    work3 = ctx.enter_context(tc.tile_pool(name="work3", bufs=2))

    wp_sb = big2.tile([128, 4, 2, C], BF16, tag="wp_sb")
    nc.sync.dma_start(wp_sb.rearrange("p a b c -> p (a b c)"), wp_d[:])

    # Pre-place the natural_log_exp_and_others table (set 6: ln+exp+square+
    # copy) so the compiler's act-table pass stops alternating between the
    # ln-only and exp-only sets inside the softmax waves.
    nc.scalar.add_instruction(mybir.InstLoadActFuncSet(
        name=nc.get_next_instruction_name(), act_func_set_id=6, ins=[], outs=[]))

    # score-chain groups: head-pair hp = heads (2hp, 2hp+1) share one
    # double-width [szk, 2, nq] tile so every wave op covers both heads.
    SPAIRS = [(hp, kc) for hp in range(2) for kc in range(5)]

    def emit_scores(q0, nq, qsb, split_waves=False):
        stiles = {}
        for (hp, kc) in SPAIRS:
            k0, szk = K_CHUNKS[kc]
            s = sm.tile([128, 2, 512], F16, tag="s", bufs=4)
            c2 = sm.tile([128, 2, 512], F16, tag="c2", bufs=2)
            for j in range(2):
                h = 2 * hp + j
                sre = psum.tile([128, 512], F32, tag="bank", name="sre")
                sim = psum.tile([128, 512], F32, tag="bank", name="sim")
                nc.tensor.matmul(sre[:szk, :nq], kre[:, h, k0:k0 + szk],
                                 qsb[:, h, :nq], start=True, stop=True)
                nc.tensor.matmul(sim[:szk, :nq], kim[:, h, k0:k0 + szk],
                                 qsb[:, h, :nq], start=True, stop=True)
                nc.scalar.activation(s[:szk, j, :nq], sre[:szk, :nq],
                                     AF.Square)
                nc.vector.tensor_copy(c2[:szk, j, :nq], sim[:szk, :nq])
            m2 = sm.tile([128, 2, 512], F16, tag="m2", bufs=2)
            nc.vector.tensor_mul(m2[:szk, :, :nq], c2[:szk, :, :nq],
                                 c2[:szk, :, :nq])
            nc.vector.tensor_add(s[:szk, :, :nq], s[:szk, :, :nq],
                                 m2[:szk, :, :nq])
            stiles[(hp, kc)] = s
        # batched waves per head-pair: Ln (one table swap) then both Exps
        # (one swap).  |a| = exp(0.5*ln(s)): the HW Sqrt LUT was measured
        # less accurate (rel 0.0201-0.0203 vs 0.0189, gate 0.02).
        # one ln wave then one exp wave per chunk (2 table swaps).  For the
        # first chunk ACT has no backlog, so run per-head-pair waves instead:
        # attn(0) can start at the half-wave mark (latency, not throughput).
        ebufs = {}
        waves = ([[p for p in SPAIRS if p[0] == hpw] for hpw in range(2)]
                 if split_waves else [SPAIRS])
        for wave in waves:
            ubs = {}
            for (hpw, kc) in wave:
                k0, szk = K_CHUNKS[kc]
                s = stiles[(hpw, kc)]
                ub = sm.tile([128, 2, 512], F32, tag="ub", bufs=10)
                nc.scalar.activation(ub[:szk, :, :nq], s[:szk, :, :nq], AF.Ln)
                ubs[(hpw, kc)] = ub
            for (hpw, kc) in wave:
                k0, szk = K_CHUNKS[kc]
                ub = ubs[(hpw, kc)]
                nc.scalar.activation(ub[:szk, :, :nq], ub[:szk, :, :nq], AF.Exp,
                                     scale=0.5)
                eb = sm.tile([128, 2, 512], BF16, tag="ebuf", bufs=12)
                nc.scalar.activation(eb[:szk, :, :nq], ub[:szk, :, :nq], AF.Exp,
                                     bias=nbias[:szk])
                ebufs[(hpw, kc)] = eb
        return ebufs

    def emit_attn_norm(q0, nq, ebufs):
        dn = psum.tile([128, 512], F32, tag="bank", name="dn")
        ots = qs.tile([128, 4, 512], BF16, tag="ots")
        for h in range(4):
            op = psum.tile([128, 512], F32, tag="bank", name=f"op{h}")
            for kc in range(5):
                k0, szk = K_CHUNKS[kc]
                ebv = ebufs[(h // 2, kc)][:szk, h % 2, :nq]
                nc.tensor.matmul(op[:, :nq], vpk[:szk, kc, h, :],
                                 ebv, start=kc == 0, stop=kc == 4)
                nc.tensor.matmul(dn[32 * h:32 * h + 1, :nq], ones_col[:szk, :],
                                 ebv, start=kc == 0, stop=kc == 4,
                                 tile_position=(0, 32 * h))
            dnr = st3.tile([1, 512], F32R, tag="dnr", bufs=2, name=f"dnr{h}")
            nc.vector.tensor_copy(dnr[:, :nq], dn[32 * h:32 * h + 1, :nq])
            if DEBUG:
                nc.sync.dma_start(dbg["dn"][h:h + 1, q0:q0 + nq],
                                  dnr[:, :nq].bitcast(F32))
            rbp = psum.tile([128, 512], F32, tag="bank", name="rbp")
            nc.tensor.matmul(rbp[:, :nq], _r(ones_row[:1, :128]), dnr[:, :nq],
                             start=True, stop=True)
            rb = sm.tile([128, 512], F32, tag="rb", bufs=1)
            nc.vector.reciprocal(rb[:, :nq], rbp[:, :nq])
            nc.vector.tensor_mul(ots[:, h, :nq], op[:, :nq], rb[:, :nq])
        return (ots,)

    def emit_proj(q0, nq, ots):
        ot = work3.tile([128, 2, 4, 512], F16, tag="ot", bufs=1)
        for cc in range(4):
            cs = slice(128 * cc, 128 * (cc + 1))
            pr = psum.tile([128, 512], F32, tag="bank", name="pjr")
            pi = psum.tile([128, 512], F32, tag="bank", name="pji")
            for h in range(4):
                nc.tensor.matmul(pr[:, :nq], wp_sb[:, h, 0, cs],
                                 ots[:, h, :nq], start=h == 0, stop=h == 3)
                nc.tensor.matmul(pi[:, :nq], wp_sb[:, h, 1, cs],
                                 ots[:, h, :nq], start=h == 0, stop=h == 3)
            nc.vector.tensor_copy(ot[:, 0, cc, :nq], pr[:, :nq])
            nc.vector.tensor_copy(ot[:, 1, cc, :nq], pi[:, :nq])
        nc.gpsimd.dma_start(outT_d[0, :, :, q0:q0 + nq].rearrange("a p b -> p a b"),
                            ot[:, 0, :, :nq])
        nc.sync.dma_start(outT_d[1, :, :, q0:q0 + nq].rearrange("a p b -> p a b"),
                          ot[:, 1, :, :nq])

    # software pipeline: PE stream per chunk j is
    #   ... scores(j+1) | attn+norm(j) | qproj(j+2) | proj(j-1) ...
    # so the PE never sits behind chunk j's softmax (ACT) chain.
    qsbs = {0: qsb0}
    ebs = {0: emit_scores(*Q_CHUNKS[0], qsbs[0], split_waves=True)}
    onos = {}
    for j, (q0, nq) in enumerate(Q_CHUNKS):
        if j + 1 < len(Q_CHUNKS):
            qsbs[j + 1] = emit_qproj(*Q_CHUNKS[j + 1])
            ebs[j + 1] = emit_scores(*Q_CHUNKS[j + 1], qsbs[j + 1])
        onos[j] = emit_attn_norm(q0, nq, ebs.pop(j))
        if j >= 1:
            emit_proj(*Q_CHUNKS[j - 1], *onos.pop(j - 1))
    emit_proj(*Q_CHUNKS[4], *onos.pop(4))

    ctx.close()


# =========================================================================
# Host side
# =========================================================================

def _f32(x):
    return np.ascontiguousarray(x, dtype=np.float32)


def _bf(x):
    return np.asarray(x, dtype=ml_dtypes.bfloat16)


def _pack_heads(w):
    """[C, 256] complex -> planes [2, C, 512]: per head h (64 cols):
    plane0 [128h:128h+64]=re, [+64:+128]=im; plane1 = [-im | re]."""
    p0 = np.zeros((w.shape[0], 512), np.float32)
    p1 = np.zeros((w.shape[0], 512), np.float32)
    for h in range(4):
        blk = w[:, 64 * h:64 * (h + 1)]
        p0[:, 128 * h:128 * h + 64] = blk.real
        p0[:, 128 * h + 64:128 * h + 128] = blk.imag
        p1[:, 128 * h:128 * h + 64] = -blk.imag
        p1[:, 128 * h + 64:128 * h + 128] = blk.real
    # -> SBUF-resident layout [128, (cj, plane, n)]
    pk = np.stack([p0, p1])                      # [2, C, 512]
    pk = pk.reshape(2, 4, 128, 512).transpose(2, 1, 0, 3)  # [128, 4, 2, 512]
    return np.ascontiguousarray(pk.reshape(128, 4 * 2 * 512))


def _pack_wp(w):
    """[256, C] complex -> [128, 4*2*C] bf16: per head h the stationaries
    [Wre_h ; -Wim_h] (-> out_re) and [Wim_h ; Wre_h] (-> out_im)."""
    arr = np.zeros((128, 4, 2, C), np.float32)
    for h in range(4):
        blk = w[64 * h:64 * (h + 1), :]
        arr[0:64, h, 0, :] = blk.real
        arr[64:128, h, 0, :] = -blk.imag
        arr[0:64, h, 1, :] = blk.imag
        arr[64:128, h, 1, :] = blk.real
    return np.ascontiguousarray(_bf(arr).reshape(128, 4 * 2 * C))


def _pack_bias(b):
    """[256] complex -> [1, 512] packed [re_h | im_h] per head."""
    out = np.zeros((1, 512), np.float32)
    for h in range(4):
        blk = b[64 * h:64 * (h + 1)]
        out[0, 128 * h:128 * h + 64] = blk.real
        out[0, 128 * h + 64:128 * h + 128] = blk.imag
    return out


def host_prep(x_re, x_im, Wq, Wkv, Wproj, bproj, sr_w, sr_b, gain, bias):
    x_re = np.asarray(x_re)
    x_im = np.asarray(x_im)
    Wq = np.asarray(Wq)
    Wkv = np.asarray(Wkv)
    Wproj = np.asarray(Wproj)
    sr_w = np.asarray(sr_w)
    sr_b = np.asarray(sr_b)
    gain = np.asarray(gain)
    bias = np.asarray(bias)

    Wkv_eff = gain[:, None] * Wkv
    bkv_full = bias @ Wkv
    Wc = sr_w.transpose(2, 3, 1, 0).reshape(4 * C, C)

    def planes3f(w):
        return np.stack([_f32(w.real), _f32(w.imag), _f32(-w.imag)])

    in_maps = []
    for core in range(8):
        b, g = core // 2, core % 2
        cols = slice(256 * g, 256 * (g + 1))
        wk_c = Wkv_eff[:, :C][:, cols] * SCALE
        wv_c = Wkv_eff[:, C:][:, cols]
        bk_c = bkv_full[:C][cols] * SCALE
        bv_c = bkv_full[C:][cols]
        xs_c = np.stack([x_re[b].T, x_im[b].T])  # [2, C, N]
        xsp = xs_c.reshape(2, C, HR, 2, HR, 2)
        xP = np.stack([xsp[:, :, :, p, :, q].reshape(2, C, NK)
                       for p in range(2) for q in range(2)], axis=1)
        xP = xP.reshape(2, 16, 128, NK).transpose(1, 2, 0, 3)
        wc3 = planes3f(Wc).reshape(3, 16, 128, 512).transpose(2, 1, 0, 3)
        m = {
            "xT": _f32(xs_c.reshape(2, 4, 128, N).transpose(0, 2, 1, 3)),
            "xP": _f32(xP.reshape(16, 128, 2 * NK)),
            "wc": _f32(wc3.reshape(128, 16 * 3 * 512)),
            "srb": np.stack([_f32(sr_b.real), _f32(sr_b.imag)]),
            "ones": np.ones((1, 512), np.float32),
            "wq": _pack_heads(Wq[:, cols]),
            "wk": _pack_heads(wk_c),
            "bk": _pack_bias(bk_c),
            "wv": np.ascontiguousarray(
                planes3f(wv_c).reshape(3, 4, 128, 256)
                .transpose(2, 1, 0, 3).reshape(128, 4 * 3 * 256)),
            "bv": np.stack([_f32(bv_c.real)[None, :], _f32(bv_c.imag)[None, :]]
                           ).reshape(2, 256),
            "wp": _pack_wp(Wproj[256 * g:256 * (g + 1), :]),
        }
        in_maps.append(m)
    return in_maps


_NC_CACHE = None


def _get_nc():
    global _NC_CACHE
    if _NC_CACHE is None:
        _NC_CACHE = build_nc()
    return _NC_CACHE


def kernel(x_re, x_im, Wq, Wkv, Wproj, bproj, sr_w, sr_b, gain, bias, H, W):
    from concourse.bass_utils import run_bass_kernel_spmd

    nc = _get_nc()
    in_maps = host_prep(x_re, x_im, Wq, Wkv, Wproj, bproj, sr_w, sr_b, gain, bias)
    res = run_bass_kernel_spmd(nc, in_maps, list(range(8)))
    bproj = np.asarray(bproj)
    out = np.zeros((B, N, C), dtype=np.complex64)
    for b in range(B):
        p0 = res.results[2 * b]["outT"].astype(np.float32).reshape(2, C, N)
        p1 = res.results[2 * b + 1]["outT"].astype(np.float32).reshape(2, C, N)
        acc = (p0[0] + p1[0]).T + 1j * (p0[1] + p1[1]).T
        out[b] = acc + bproj[None, :]
    return out
